# revision 1
# baseline (speedup 1.0000x reference)
"""Trainium2 Bass kernel for nn_CAttention (channel attention).

Reference computation (per batch b):
    k      = einsum('cit,i->ct', x[b], alpha)          # [C, T]
    scores = k @ W @ k.T                               # [C, C]
    att    = softmax(scores, axis=-1)
    out[b] = att @ x[b].reshape(C, N*T)                # [C, N*T]

Shapes (hardcoded): x [64, 256, 307, 12] f32, W [12, 12], alpha [307].
Sharding: data-parallel over batch B across 8 cores (8 batches/core);
W and alpha replicated.

The kernel is HBM-DMA bound: 30.2 MB in + 30.2 MB out per core across
16 DMA engines capped at ~24 GB/s each gives a ~158 us floor.  The
design keeps those engines saturated end to end:
 - x loads are issued from the SP (sync) sequencer, output stores from
   the Activation sequencer — separate hardware-DGE rings, so a load
   issue that blocks on x-buffer reuse can never head-of-line block a
   ready store.
 - Output rows are staged as full [128, 3684] SBUF rows and stored with
   one DMA per c-chunk: 14.7 KB per-partition descriptors.
 - Each x c-chunk load is split in two at the node boundary i=154 so
   the k-pooling starts while the rest of the batch still streams.
 - k = sum_i alpha_i * x[:, i, :]: the alpha multiply writes t-major
   scratch split across Pool (i<225) and DVE (the rest), then DVE
   reduces both scratches with unit-stride reduce_sums (free-axis
   reduces are DVE-only; they measure ~3x cheaper per element than
   tensor_tensor, and Pool's bigger mul share rebalances the load).
   DVE carries nothing else — any other op in its stream would queue
   the next batch behind ~12 us of k work.
 - The big matmul runs in float32r (fp32 truncated to 11 mantissa bits
   at the PE) which streams 1 cycle/column like bf16 for >=256-wide
   moving tiles; x is DMA'd into a float32r-typed tile so the k-path
   reads the same bits as full fp32.
 - Softmax needs no transpose: scoresT [d, c] is computed directly,
   exp() writes attT in place as the big-matmul stationary, and the
   denominator comes from a ones-column appended to x (written by a
   tiny ACT copy, so no k-path engine ever waits on x-buffer reuse) —
   the big matmul emits sum_d exp(scores[c,d]) as an extra output
   column and the normalization folds into the PSUM->SBUF copy.
   exp() skips max-subtraction: |scores| <= ~30 here.
 - PSUM: all 8 banks at once per c-chunk, so each c-chunk is 16
   back-to-back matmuls under two LDWEIGHTS (walrus ldw-opt elides the
   rest) and the PE stream stays dense enough to ramp its p-state.
"""

from contextlib import ExitStack

import numpy as np

import concourse.bass as bass
import concourse.bass_utils as _bass_utils
import concourse.tile as tile
from concourse import bacc, mybir
from concourse.bass import ts
from concourse.bass_utils import run_bass_kernel_spmd
from concourse.masks import make_identity

B, C, N, T = 64, 256, 307, 12
NCORES = 8
B_LOC = B // NCORES          # 8 batches per core
F = N * T                    # 3684 flattened free dim
FW = F + 2                   # + ones col (softmax denominator) + pad col
P = 128                      # partitions
CC = C // P                  # 2 c-chunks
NS = 225                     # node split: Pool engine muls i<225, DVE the rest
FA = NS * T                  # 2700 columns in the first load chunk

# f-tiles of the big matmul: one PSUM bank each, all 8 at once per
# c-chunk, so each c-chunk runs 16 back-to-back matmuls with only two
# LDWEIGHTS (ldw-opt).  The tile covering the appended ones-column
# (denominator at output col 3684 -> offset 100) goes first so the
# normalizer is ready before any PSUM->SBUF copy.
_FTILES = [(3584, 102)] + [(i * 512, 512) for i in range(7)]
_DEN_OFF = 100               # denominator column offset inside tile 0

_DT = mybir.dt.float32
_R = mybir.dt.float32r


def _enable_ldw_opt():
    """Compile with --enable-ldw-opt=true so walrus elides LDWEIGHTS for
    consecutive matmuls sharing the stationary operand.  bass_utils
    hardcodes false; float32r cannot use standalone ldweights, so this
    is the only way to amortize 4-byte weight loads."""
    if getattr(_bass_utils, "_ldw_opt_patched", False):
        return
    orig = _bass_utils.bir_verify_and_optimise

    def patched(tmpdir, inp="bir.json", outp="file.neff", arch=None, *, dve_root=None):
        real_run = _bass_utils.run_command

        def run_hook(argv, **kw):
            argv = [
                "--enable-ldw-opt=true" if a == "--enable-ldw-opt=false" else a
                for a in argv
            ]
            return real_run(argv, **kw)

        _bass_utils.run_command = run_hook
        try:
            return orig(tmpdir, inp, outp, arch, dve_root=dve_root)
        finally:
            _bass_utils.run_command = real_run

    _bass_utils.bir_verify_and_optimise = patched
    _bass_utils._ldw_opt_patched = True


def _emit_core_kernel(tc, x_ap, w_ap, alpha_ap, out_ap):
    """Emit the per-core program. x_ap/out_ap: [B_LOC, C, N, T] DRAM."""
    nc = tc.nc
    ctx = ExitStack()

    x_flat = x_ap.rearrange("b c i t -> b c (i t)")      # [B_LOC, C, F]
    out_flat = out_ap.rearrange("b c i t -> b c (i t)")  # [B_LOC, C, F]

    consts = ctx.enter_context(tc.tile_pool(name="consts", bufs=1))
    xpool = ctx.enter_context(tc.tile_pool(name="x", bufs=4))
    xgpool = ctx.enter_context(tc.tile_pool(name="xg", bufs=2))
    xvpool = ctx.enter_context(tc.tile_pool(name="xv", bufs=2))
    kpool = ctx.enter_context(tc.tile_pool(name="k", bufs=6))
    ktpool = ctx.enter_context(tc.tile_pool(name="kt", bufs=4))
    attpool = ctx.enter_context(tc.tile_pool(name="att", bufs=3))
    outpool = ctx.enter_context(tc.tile_pool(name="out", bufs=2))
    rpool = ctx.enter_context(tc.tile_pool(name="rinv", bufs=4))
    # single shared PSUM pool: every tile one full bank, 8 banks total
    psum = ctx.enter_context(tc.tile_pool(name="psum", bufs=8, space="PSUM"))

    # Constants: identity for PE transpose, alpha broadcast, W, ones.
    ident = consts.tile([P, P], _DT)
    make_identity(nc, ident)
    alpha_row = consts.tile([P, N], _DT)
    nc.gpsimd.dma_start(out=alpha_row, in_=alpha_ap[None, :].to_broadcast([P, N]))
    w_sb = consts.tile([T, T], _DT)
    nc.gpsimd.dma_start(out=w_sb, in_=w_ap)
    # ones moving operand for the softmax-denominator matmul columns
    ones_c = consts.tile([P, 2], _DT)
    nc.gpsimd.memset(ones_c, 1.0)

    def phase1a(b):
        """Load x[b] (split loads); k via Pool/DVE t-major mul + reduce.

        The alpha multiply writes t-major scratch (strided writes cost
        the same as natural-layout ones on these engines) so the
        i-reduction is a single unit-stride reduce_sum per engine —
        reduces measure ~3x cheaper per element than tensor_tensor."""
        x_t = xpool.tile([P, CC, FW], _R, tag="x")
        for cc in range(CC):
            nc.sync.dma_start(
                out=x_t[:, cc, :FA], in_=x_flat[b, ts(cc, P), :FA].bitcast(_R)
            )
            nc.sync.dma_start(
                out=x_t[:, cc, FA:F], in_=x_flat[b, ts(cc, P), FA:].bitcast(_R)
            )
            # ones-columns for the denominator; ACT copy (with fp32->
            # fp32r rounding on write, exact for 1.0) so no engine on
            # the k critical path ever waits on x-buffer reuse.
            nc.scalar.copy(out=x_t[:, cc, F:FW], in_=ones_c)

        k_c = kpool.tile([P, CC, T], _DT, tag="k")
        return {"x_t": x_t, "k_c": k_c}

    def phase1a_k(st, cc):
        """k for one c-chunk: separate t-major scratches per engine
        (concurrent writers to one tile contend on SBUF write ports);
        DVE reduces both (free-axis reduces are DVE-only; Pool's bigger
        mul share rebalances the load)."""
        x_t, k_c = st["x_t"], st["k_c"]
        xg = xgpool.tile([P, T, NS], _DT, tag="xg")
        xv = xvpool.tile([P, T, N - NS], _DT, tag="xv")
        ka = kpool.tile([P, 2, T], _DT, tag="ka")
        x_cc = x_t[:, cc, :F].bitcast(_DT).rearrange("p (i t) -> p i t", t=T)
        nc.gpsimd.tensor_mul(
            xg.rearrange("p t i -> p i t"),
            x_cc[:, :NS, :],
            alpha_row[:, :NS, None].to_broadcast([P, NS, T]),
        )
        nc.vector.tensor_mul(
            xv.rearrange("p t i -> p i t"),
            x_cc[:, NS:, :],
            alpha_row[:, NS:, None].to_broadcast([P, N - NS, T]),
        )
        nc.vector.reduce_sum(out=ka[:, 0, :], in_=xg, axis=mybir.AxisListType.X)
        nc.vector.reduce_sum(out=ka[:, 1, :], in_=xv, axis=mybir.AxisListType.X)
        nc.vector.tensor_add(k_c[:, cc, :], ka[:, 0, :], ka[:, 1, :])

    # The scores chain kT -> kWT -> scoresT -> attT is a 6-hop
    # PE/ACT ping-pong (~6 us of latency).  Its pieces are emitted
    # between / after the two big-matmul c-chunk blocks of the previous
    # batch, so each cross-engine hop has a few microseconds of queued
    # PE work in front of it and the latency hides inside the period.
    def phase1b_pre(st):
        """Transpose k -> kT (both c-chunks into one PSUM bank)."""
        k_c = st["k_c"]
        ps_kt = psum.tile([P, 512], _DT, tag="ps")
        nc.tensor.transpose(ps_kt[:T, 0:P], k_c[:, 0, :], ident)
        nc.tensor.transpose(ps_kt[:T, P:C], k_c[:, 1, :], ident)
        kt_sb = ktpool.tile([T, C], _DT, tag="kt")
        nc.scalar.copy(out=kt_sb, in_=ps_kt[:T, :C])
        st["kt_sb"] = kt_sb

    def phase1b_mid(st):
        """kWT[s, c] = sum_t W[t, s] kT[t, c]."""
        ps_kwt = psum.tile([P, 512], _DT, tag="ps")
        nc.tensor.matmul(
            ps_kwt[:T, :C], lhsT=w_sb, rhs=st["kt_sb"], start=True, stop=True
        )
        kwt_sb = ktpool.tile([T, C], _DT, tag="kwt")
        nc.scalar.copy(out=kwt_sb, in_=ps_kwt[:T, :C])
        st["kwt_sb"] = kwt_sb

    def phase1b_sc(st):
        """scoresT[d, c] = sum_s kT[s, d] kWT[s, c]  (= scores[c, d])."""
        ps_sc = psum.tile([P, 512], _DT, tag="ps")
        for dc in range(CC):
            nc.tensor.matmul(
                ps_sc[:, ts(dc, C)],
                lhsT=st["kt_sb"][:, ts(dc, P)],
                rhs=st["kwt_sb"],
                start=True,
                stop=True,
            )
        st["ps_sc"] = ps_sc

    def phase1b_exp(st):
        """attT = exp(scoresT), written directly as fp32r matmul weights."""
        att_t = attpool.tile([P, CC, C], _R, tag="attT")
        for dc in range(CC):
            nc.scalar.activation(
                out=att_t[:, dc, :],
                in_=st["ps_sc"][:, ts(dc, C)],
                func=mybir.ActivationFunctionType.Exp,
            )
        st["att_t"] = att_t

    def phase2_mm(st, cc):
        """Big matmul out[c, f] (+ denominator column) for one c-chunk:
        all 8 PSUM banks at once, 16 back-to-back matmuls with two
        LDWEIGHTS."""
        x_t, att_t = st["x_t"], st["att_t"]
        pss = [
            psum.tile([P, 512], _DT, tag="ps", name=f"ps_o{i}")
            for i in range(len(_FTILES))
        ]
        for dc in range(CC):
            for (f0, fsz), pt in zip(_FTILES, pss):
                nc.tensor.matmul(
                    pt[:, :fsz],
                    lhsT=att_t[:, dc, ts(cc, P)],
                    rhs=x_t[:, dc, f0 : f0 + fsz],
                    start=(dc == 0),
                    stop=(dc == CC - 1),
                )
        return pss

    def phase2_fin(b, st, cc, pss):
        """Normalize and store one c-chunk.  The reciprocal must run on
        DVE (ACT's Reciprocal is banned for accuracy), so the step loop
        emits each _fin at a point where DVE's k pipeline has naturally
        caught up past the denominator matmul — the recip never makes
        the k work queue behind it."""
        rinv = rpool.tile([P, 1], _DT, tag="rinv")
        o_row = outpool.tile([P, F], _DT, tag="o")
        # output col 3684 (offset 100 of tile 0) = sum_d exp(scores)
        nc.vector.reciprocal(out=rinv, in_=pss[0][:, _DEN_OFF : _DEN_OFF + 1])
        for (f0, fsz), pt in zip(_FTILES, pss):
            osz = min(fsz, F - f0)  # drop the ones-columns
            nc.scalar.mul(
                out=o_row[:, f0 : f0 + osz], in_=pt[:, :osz], mul=rinv
            )
        # one full-row store per c-chunk, issued from the ACT ring so
        # it can never queue behind a blocked x-load issue.
        nc.scalar.dma_start(out=out_flat[b, ts(cc, P), :], in_=o_row)

    # Three-stage software pipeline.  Per step s, the emission order
    # threads three batches together so every cross-engine hop has
    # queued work in front of it:
    #   - batch s-3's two big-matmul blocks + normalize/store,
    #   - batch s-2's scores chain between and after them,
    #   - batch s's loads and the two k chains, with c-chunk 1's k
    #     emitted last so DVE reaches each reciprocal exactly when its
    #     denominator is ready and the k work never queues behind it.
    states = {}
    for s in range(B_LOC + 3):
        st3 = states.pop(s - 3, None)
        st2 = states.get(s - 2) if 0 <= s - 2 < B_LOC else None
        ps0 = phase2_mm(st3, 0) if st3 is not None else None
        if ps0 is not None:
            phase2_fin(s - 3, st3, 0, ps0)
        if st2 is not None:
            phase1b_pre(st2)
        ps1 = phase2_mm(st3, 1) if st3 is not None else None
        if s < B_LOC:
            states[s] = phase1a(s)
            phase1a_k(states[s], 0)
        if ps1 is not None:
            phase2_fin(s - 3, st3, 1, ps1)
        if st2 is not None:
            phase1b_mid(st2)
            phase1b_sc(st2)
            phase1b_exp(st2)
        if s < B_LOC:
            phase1a_k(states[s], 1)
    ctx.close()


_CACHED_NC = None


def _build():
    global _CACHED_NC
    if _CACHED_NC is not None:
        return _CACHED_NC
    _enable_ldw_opt()
    nc = bacc.Bacc("TRN2", target_bir_lowering=False, debug=False, num_devices=NCORES)
    x_d = nc.dram_tensor("x", [B_LOC, C, N, T], _DT, kind="ExternalInput").ap()
    w_d = nc.dram_tensor("W", [T, T], _DT, kind="ExternalInput").ap()
    a_d = nc.dram_tensor("alpha", [N], _DT, kind="ExternalInput").ap()
    o_d = nc.dram_tensor("out", [B_LOC, C, N, T], _DT, kind="ExternalOutput").ap()
    with tile.TileContext(nc) as tc:
        _emit_core_kernel(tc, x_d, w_d, a_d, o_d)
    nc.compile()
    _CACHED_NC = nc
    return nc


def run(x, W, alpha, trace=False, **spmd_kwargs):
    """Run on 8 cores; returns (full output [B,C,N,T], BassKernelResults)."""
    x = np.ascontiguousarray(np.asarray(x, dtype=np.float32))
    W = np.ascontiguousarray(np.asarray(W, dtype=np.float32))
    alpha = np.ascontiguousarray(np.asarray(alpha, dtype=np.float32))
    assert x.shape == (B, C, N, T) and W.shape == (T, T) and alpha.shape == (N,)

    nc = _build()
    in_maps = [
        {"x": x[i * B_LOC : (i + 1) * B_LOC], "W": W, "alpha": alpha}
        for i in range(NCORES)
    ]
    res = run_bass_kernel_spmd(
        nc, in_maps, core_ids=list(range(NCORES)), trace=trace, **spmd_kwargs
    )
    out = np.concatenate([r["out"] for r in res.results], axis=0)
    return out, res


def kernel(x, W, alpha):
    out, _ = run(x, W, alpha)
    return out



# revision 5
# speedup vs baseline: 1.5036x; 1.5036x over previous
"""Trainium2 Bass kernel for nn_CAttention (channel attention).

Reference computation (per batch b):
    k      = einsum('cit,i->ct', x[b], alpha)          # [C, T]
    scores = k @ W @ k.T                               # [C, C]
    att    = softmax(scores, axis=-1)
    out[b] = att @ x[b].reshape(C, N*T)                # [C, N*T]

Shapes (hardcoded): x [64, 256, 307, 12], W [12, 12], alpha [307].
Sharding: data-parallel over batch B across 8 cores (8 batches/core);
W and alpha replicated.

The kernel is HBM-DMA bound, so both x and out travel as fp16 (host
casts x f32->fp16 on the way in and out fp16->f32 on the way back):
15.1 MB in + 15.1 MB out per core => ~88 us floor at the ~341 GB/s
effective per-core DMA rate.  End-to-end fp16/bf16 error is ~1.7e-3 L2
(scores stay f32; exp(scores) can reach e^31 so the unnormalized
attention weights are stored bf16, whose range absorbs it).

Per-engine budget per batch (target period ~11 us):
 - DMA: one 1.9 MB load (SP ring) + two 0.95 MB stores (ACT ring).
 - DVE owns the k-pooling: one packed-2x fp16 multiply by a
   materialized alpha[i]-per-(i,t) row, then an in-place fold tree
   (307 -> 154 -> 77 -> 39 -> 20 i's) of packed-2x adds, then one
   strided reduce_sum of the last 20 i's.  ~4.9 us/chunk.  TensorReduce
   and TensorTensorReduce have no DVE fast modes (1 elem/cyc) and
   GpSimd tensor ops run at 0.42 efficiency AND contend with DVE's
   tensor_tensor port, so this all-DVE packed shape beats every
   Pool-assisted split of the baseline.
 - PE: big matmul att^T[d,c] x[d,f] streams fp16 at 1 col/cyc (the
   f32r baseline was moving-operand-port-limited); scores chain f32.
 - ACT: PSUM->SBUF drains fused with the 1/den normalize (rinv is
   per-partition), exp, kT/kWT evacuations, ones columns.
 - Softmax denominator comes from a ones-column appended to x; each
   PSUM f-tile runs its two contraction matmuls back-to-back with the
   denominator tile first, so the DVE reciprocal fires ~0.5 us into
   the chunk and never stalls the pooling stream behind it.

Three-stage pipeline with one-step load prefetch: step s issues
load(s+1), then mm+fin(s-2) / scores(s-1) / pooling(s) interleaved so
the DVE order is recip(c0), pool(c0), recip(c1), pool(c1).
"""

from contextlib import ExitStack

import numpy as np

import concourse.bass as bass
import concourse.bass_utils as _bass_utils
import concourse.tile as tile
from concourse import bacc, mybir
from concourse.bass import ts
from concourse.bass_utils import run_bass_kernel_spmd
from concourse.masks import make_identity

B, C, N, T = 64, 256, 307, 12
NCORES = 8
B_LOC = B // NCORES          # 8 batches per core
F = N * T                    # 3684 flattened free dim
FW = F + 2                   # + ones col (softmax denominator) + pad col
P = 128                      # partitions
CC = C // P                  # 2 c-chunks

# f-tiles of the big matmul: one PSUM bank each, all 8 banks per
# c-chunk.  The tile covering the appended ones-column (denominator at
# output col 3684 -> offset 100) goes first and runs both contraction
# matmuls back-to-back so the normalizer is ready immediately.
_FTILES = [(3584, 102)] + [(i * 512, 512) for i in range(7)]
_DEN_OFF = 100               # denominator column offset inside tile 0

# In-place fold tree for the i-reduction, in element offsets (i*T).
# Each level folds src range [s0, s1) onto dst [d0, d0+(s1-s0)); all
# ranges are 4B-aligned with even element counts so fp16 tensor_add
# runs in packed 2x_1p mode.  Afterwards i in [0, 20) remains.
_FOLDS = [
    (12, 1848, 3684),   # i[154..307) -> i[1..154)
    (0, 924, 1848),     # i[77..154)  -> i[0..77)
    (12, 468, 924),     # i[39..77)   -> i[1..39)
    (12, 240, 468),     # i[20..39)   -> i[1..20)
]
_REM = 20                    # i's remaining for the final reduce_sum

_F32 = mybir.dt.float32
_F16 = mybir.dt.bfloat16  # TEMP: all-bf16 A/B test
_BF16 = mybir.dt.bfloat16


def _enable_ldw_opt():
    """Compile with --enable-ldw-opt=true so walrus elides LDWEIGHTS for
    consecutive matmuls sharing the stationary operand."""
    if getattr(_bass_utils, "_ldw_opt_patched", False):
        return
    orig = _bass_utils.bir_verify_and_optimise

    def patched(tmpdir, inp="bir.json", outp="file.neff", arch=None, *, dve_root=None):
        real_run = _bass_utils.run_command

        def run_hook(argv, **kw):
            argv = [
                "--enable-ldw-opt=true" if a == "--enable-ldw-opt=false" else a
                for a in argv
            ]
            return real_run(argv, **kw)

        _bass_utils.run_command = run_hook
        try:
            return orig(tmpdir, inp, outp, arch, dve_root=dve_root)
        finally:
            _bass_utils.run_command = real_run

    _bass_utils.bir_verify_and_optimise = patched
    _bass_utils._ldw_opt_patched = True


def _emit_core_kernel(tc, x_ap, w_ap, alpha_ap, out_ap):
    """Emit the per-core program. x_ap/out_ap: [B_LOC, C, N, T] DRAM fp16."""
    nc = tc.nc
    ctx = ExitStack()

    x_flat = x_ap.rearrange("b c i t -> b c (i t)")      # [B_LOC, C, F]
    out_flat = out_ap.rearrange("b c i t -> b c (i t)")  # [B_LOC, C, F]

    consts = ctx.enter_context(tc.tile_pool(name="consts", bufs=1))
    xpool = ctx.enter_context(tc.tile_pool(name="x", bufs=4))
    prodpool = ctx.enter_context(tc.tile_pool(name="prod", bufs=2))
    kpool = ctx.enter_context(tc.tile_pool(name="k", bufs=3))
    ktpool = ctx.enter_context(tc.tile_pool(name="kt", bufs=3))
    attpool = ctx.enter_context(tc.tile_pool(name="att", bufs=3))
    outpool = ctx.enter_context(tc.tile_pool(name="out", bufs=2))
    rpool = ctx.enter_context(tc.tile_pool(name="rinv", bufs=4))
    # single shared PSUM pool: every tile one full bank, 8 banks total
    psum = ctx.enter_context(tc.tile_pool(name="psum", bufs=8, space="PSUM"))

    # Constants: identity for PE transpose, W, ones columns, and the
    # alpha row expanded to one fp16 weight per (i, t) column so the
    # pooling multiply is a unit-stride packed tensor_tensor.
    ident = consts.tile([P, P], _F32)
    make_identity(nc, ident)
    w_sb = consts.tile([T, T], _F32)
    nc.gpsimd.dma_start(out=w_sb, in_=w_ap)
    alpha_row = consts.tile([P, N], _F32)
    nc.gpsimd.dma_start(out=alpha_row, in_=alpha_ap[None, :].to_broadcast([P, N]))
    alpha_full = consts.tile([P, F], _F16)
    nc.vector.tensor_copy(
        alpha_full.rearrange("p (i t) -> p i t", t=T),
        alpha_row[:, :, None].to_broadcast([P, N, T]),
    )
    ones_c = consts.tile([P, CC, FW - F], _F16)
    nc.gpsimd.memset(ones_c, 1.0)

    def phase_load(b):
        """One DMA for the whole batch (1.9 MB, SP ring) + ones cols."""
        x_t = xpool.tile([P, CC, FW], _F16, tag="x")
        nc.sync.dma_start(
            out=x_t[:, :, :F], in_=x_flat[b].rearrange("(cc p) f -> p cc f", p=P)
        )
        nc.scalar.copy(out=x_t[:, :, F:FW], in_=ones_c)
        k_c = kpool.tile([P, CC, T], _F32, tag="k")
        return {"x_t": x_t, "k_c": k_c}

    def phase_pool(st, cc):
        """k for one c-chunk, entirely on DVE in packed 2x mode."""
        prod = prodpool.tile([P, F], _F16, tag="prod")
        nc.vector.tensor_mul(prod, st["x_t"][:, cc, :F], alpha_full)
        for d0, s0, s1 in _FOLDS:
            n = s1 - s0
            nc.vector.tensor_add(
                prod[:, d0 : d0 + n], prod[:, d0 : d0 + n], prod[:, s0:s1]
            )
        nc.vector.reduce_sum(
            out=st["k_c"][:, cc, :],
            in_=prod[:, : _REM * T].rearrange("p (i t) -> p t i", t=T),
            axis=mybir.AxisListType.X,
        )

    def phase_scores_pre(st):
        """Transpose k -> kT (both c-chunks into one PSUM bank)."""
        ps_kt = psum.tile([P, 512], _F32, tag="ps")
        nc.tensor.transpose(ps_kt[:T, 0:P], st["k_c"][:, 0, :], ident)
        nc.tensor.transpose(ps_kt[:T, P:C], st["k_c"][:, 1, :], ident)
        kt_sb = ktpool.tile([T, C], _F32, tag="kt")
        nc.scalar.copy(out=kt_sb, in_=ps_kt[:T, :C])
        st["kt_sb"] = kt_sb

    def phase_scores_mid(st):
        """kWT, scoresT, exp -> unnormalized attT as bf16 weights."""
        ps_kwt = psum.tile([P, 512], _F32, tag="ps")
        nc.tensor.matmul(
            ps_kwt[:T, :C], lhsT=w_sb, rhs=st["kt_sb"], start=True, stop=True
        )
        kwt_sb = ktpool.tile([T, C], _F32, tag="kwt")
        nc.scalar.copy(out=kwt_sb, in_=ps_kwt[:T, :C])
        ps_sc = psum.tile([P, 512], _F32, tag="ps")
        for dc in range(CC):
            nc.tensor.matmul(
                ps_sc[:, ts(dc, C)],
                lhsT=st["kt_sb"][:, ts(dc, P)],
                rhs=kwt_sb,
                start=True,
                stop=True,
            )
        att_t = attpool.tile([P, CC, C], _BF16, tag="attT")
        for dc in range(CC):
            nc.scalar.activation(
                out=att_t[:, dc, :],
                in_=ps_sc[:, ts(dc, C)],
                func=mybir.ActivationFunctionType.Exp,
            )
        st["att_t"] = att_t

    def phase_mm(st, cc):
        """Big matmul for one c-chunk: att (bf16 stationary) x x (fp16
        moving).  The denominator tile runs its two contraction matmuls
        first; the remaining tiles group by contraction chunk so
        ldw-opt elides their LDWEIGHTS."""
        x_t, att_t = st["x_t"], st["att_t"]
        pss = [
            psum.tile([P, 512], _F32, tag="ps", name=f"ps_o{i}")
            for i in range(len(_FTILES))
        ]

        def mm(dc, i):
            f0, fsz = _FTILES[i]
            nc.tensor.matmul(
                pss[i][:, :fsz],
                lhsT=att_t[:, dc, ts(cc, P)],
                rhs=x_t[:, dc, f0 : f0 + fsz],
                start=(dc == 0),
                stop=(dc == CC - 1),
            )

        mm(0, 0)
        mm(1, 0)
        for dc in range(CC):
            for i in range(1, len(_FTILES)):
                mm(dc, i)
        return pss

    def phase_fin(b, st, cc, pss):
        """Reciprocal (DVE), normalize-drain (ACT), store (ACT ring)."""
        rinv = rpool.tile([P, 1], _F32, tag="rinv")
        nc.vector.reciprocal(out=rinv, in_=pss[0][:, _DEN_OFF : _DEN_OFF + 1])
        if cc == 0:
            st["o_t"] = outpool.tile([P, CC, F], _F16, tag="o", name="o_t")
        o_t = st["o_t"]
        for (f0, fsz), pt in zip(_FTILES, pss):
            osz = min(fsz, F - f0)  # drop the ones-columns
            nc.scalar.mul(out=o_t[:, cc, f0 : f0 + osz], in_=pt[:, :osz], mul=rinv)
        nc.scalar.dma_start(out=out_flat[b, ts(cc, P), :], in_=o_t[:, cc, :])

    # Three-stage pipeline, one-step load prefetch.  DVE program order
    # per step: recip(s-2,c0), pool(s,c0), recip(s-2,c1), pool(s,c1).
    states = {}
    for s in range(-1, B_LOC + 2):
        if 0 <= s + 1 < B_LOC:
            states[s + 1] = phase_load(s + 1)
        st2 = states.get(s - 2)
        st1 = states.get(s - 1)
        st0 = states.get(s)
        if st2 is not None:
            phase_fin(s - 2, st2, 0, phase_mm(st2, 0))
        if st1 is not None:
            phase_scores_pre(st1)
        if st0 is not None:
            phase_pool(st0, 0)
        if st2 is not None:
            phase_fin(s - 2, st2, 1, phase_mm(st2, 1))
            states.pop(s - 2)
        if st1 is not None:
            phase_scores_mid(st1)
        if st0 is not None:
            phase_pool(st0, 1)
    ctx.close()


_CACHED_NC = None


def _build():
    global _CACHED_NC
    if _CACHED_NC is not None:
        return _CACHED_NC
    # NOTE: no ldw-opt here (unlike the f32r baseline): walrus rejects the
    # standalone InstLdweights that 16-bit stationaries emit when
    # --enable-ldw-opt=true, and bf16 weight loads are cheap anyway.
    nc = bacc.Bacc("TRN2", target_bir_lowering=False, debug=False, num_devices=NCORES)
    x_d = nc.dram_tensor("x", [B_LOC, C, N, T], _F16, kind="ExternalInput").ap()
    w_d = nc.dram_tensor("W", [T, T], _F32, kind="ExternalInput").ap()
    a_d = nc.dram_tensor("alpha", [N], _F32, kind="ExternalInput").ap()
    o_d = nc.dram_tensor("out", [B_LOC, C, N, T], _F16, kind="ExternalOutput").ap()
    with tile.TileContext(nc) as tc:
        _emit_core_kernel(tc, x_d, w_d, a_d, o_d)
    nc.compile()
    _CACHED_NC = nc
    return nc


def run(x, W, alpha, trace=False, **spmd_kwargs):
    """Run on 8 cores; returns (full output [B,C,N,T], BassKernelResults)."""
    x = np.ascontiguousarray(np.asarray(x, dtype=np.float32))
    W = np.ascontiguousarray(np.asarray(W, dtype=np.float32))
    alpha = np.ascontiguousarray(np.asarray(alpha, dtype=np.float32))
    assert x.shape == (B, C, N, T) and W.shape == (T, T) and alpha.shape == (N,)

    import ml_dtypes
    x16 = x.astype(ml_dtypes.bfloat16)
    nc = _build()
    in_maps = [
        {"x": x16[i * B_LOC : (i + 1) * B_LOC], "W": W, "alpha": alpha}
        for i in range(NCORES)
    ]
    res = run_bass_kernel_spmd(
        nc, in_maps, core_ids=list(range(NCORES)), trace=trace, **spmd_kwargs
    )
    out = np.concatenate([r["out"] for r in res.results], axis=0).astype(np.float32)
    return out, res


def kernel(x, W, alpha):
    out, _ = run(x, W, alpha)
    return out


# revision 9
# speedup vs baseline: 1.6034x; 1.0664x over previous
"""Trainium2 Bass kernel for nn_CAttention (channel attention).

Reference computation (per batch b):
    k      = einsum('cit,i->ct', x[b], alpha)          # [C, T]
    scores = k @ W @ k.T                               # [C, C]
    att    = softmax(scores, axis=-1)
    out[b] = att @ x[b].reshape(C, N*T)                # [C, N*T]

Shapes (hardcoded): x [64, 256, 307, 12], W [12, 12], alpha [307].
Sharding: data-parallel over batch B across 8 cores (8 batches/core);
W and alpha replicated.

The kernel is HBM-DMA bound, so x and out travel as fp16 (host casts
f32->fp16 in, fp16->f32 back): 15.1 MB in + 15.1 MB out per core =>
~85 us floor at the ~350 GB/s effective per-core DMA rate.  fp16
everywhere measures ~1.5e-3 L2 end-to-end (bf16 weights measured
1.3e-2 on HW - too close to the gate; and NOTE: a mixed bf16 x fp16
matmul FAULTS the device, NRT_EXEC_UNIT_UNRECOVERABLE, so every
matmul here keeps both operands the same dtype).

exp(scores) reaches e^31 which overflows fp16, so attention weights
are normalized BEFORE they become matmul weights: scores are computed
in row orientation [c-part, d-free] (stationary kWT chunk x moving
kT), exp on ACT emits f32 weights plus the softmax denominator via
accum_out in the same pass, DVE takes one reciprocal, ACT's normalize
multiply (per-partition rinv) emits fp16 weights <= 1, and four PE
transposes flip them to the [d-part, c] stationary layout.

Per-engine budget per batch (target period ~10.5 us):
 - DMA: one 1.9 MB batch load (SP ring) + two 0.95 MB stores (ACT).
 - DVE ~9.9: owns k-pooling: packed-2x fp16 multiply by a materialized
   alpha-per-(i,t) row, in-place packed fold tree 307->154->77->39->20
   i's, one strided reduce_sum.  (TensorReduce/TensorTensorReduce have
   no DVE fast modes, and GpSimd tensor ops run at 0.42 efficiency and
   contend for DVE's tensor_tensor port, so all-DVE packed is optimal;
   the Pool engine stays idle on purpose.)  Plus the one reciprocal.
 - ACT ~9.9: PSUM drains as 3 copies per c-chunk (1536/1536/612 cols
   spanning 3/3/2 banks - fewer, larger copies; ACT gets no 16-bit
   speedup so element count and instruction count are what matter),
   kT/kWT evacuations (casting to fp16), exp+accum, normalize.
 - PE ~7.5: big matmul all-fp16 at 1 col/cyc, fp16 scores matmuls.
 - PSUM: two 3-bank tiles (big-matmul groups, ping-pong) + two 1-bank
   tiles (scores chain) = exactly 8 banks, no pool conflicts.

Three-stage pipeline with one-step load prefetch; the emission order
interleaves engines so every cross-engine hop has queued work and the
DVE order is pool(c0), pool(c1), recip.
"""

from contextlib import ExitStack

import numpy as np

import concourse.bass as bass
import concourse.bass_utils as _bass_utils
import concourse.tile as tile
from concourse import bacc, mybir
from concourse.bass import ts
from concourse.bass_utils import run_bass_kernel_spmd
from concourse.masks import make_identity

B, C, N, T = 64, 256, 307, 12
NCORES = 8
B_LOC = B // NCORES          # 8 batches per core
F = N * T                    # 3684 flattened free dim
P = 128                      # partitions
CC = C // P                  # 2 c-chunks

# Big-matmul PSUM groups per c-chunk: (f0, size, [sub-tile sizes]).
# Each sub-tile is one matmul dest (<=512 cols, bank-aligned inside a
# 3-bank group tile); each group drains with a single ACT copy.
_GROUPS = [
    (0, 1536, (512, 512, 512)),
    (1536, 1536, (512, 512, 512)),
    (3072, 612, (512, 100)),
]

# In-place fold tree for the i-reduction, in element offsets (i*T).
# Each level folds src range [s0, s1) onto dst [d0, d0+(s1-s0)); all
# ranges are 4B-aligned with even element counts so fp16 tensor_add
# runs in packed 2x_1p mode.  Afterwards i in [0, 20) remains.
_FOLDS = [
    (12, 1848, 3684),   # i[154..307) -> i[1..154)
    (0, 924, 1848),     # i[77..154)  -> i[0..77)
    (12, 468, 924),     # i[39..77)   -> i[1..39)
    (12, 240, 468),     # i[20..39)   -> i[1..20)
]
_REM = 20                    # i's remaining for the final reduce_sum

_F32 = mybir.dt.float32
_F16 = mybir.dt.float16


def _emit_core_kernel(tc, x_ap, w_ap, alpha_ap, out_ap):
    """Emit the per-core program. x_ap/out_ap: [B_LOC, C, N, T] DRAM fp16."""
    nc = tc.nc
    ctx = ExitStack()

    x_flat = x_ap.rearrange("b c i t -> b c (i t)")      # [B_LOC, C, F]
    out_flat = out_ap.rearrange("b c i t -> b c (i t)")  # [B_LOC, C, F]

    consts = ctx.enter_context(tc.tile_pool(name="consts", bufs=1))
    xpool = ctx.enter_context(tc.tile_pool(name="x", bufs=4))
    prodpool = ctx.enter_context(tc.tile_pool(name="prod", bufs=2))
    kpool = ctx.enter_context(tc.tile_pool(name="k", bufs=3))
    ktpool = ctx.enter_context(tc.tile_pool(name="kt", bufs=3))
    apool = ctx.enter_context(tc.tile_pool(name="att32", bufs=2))
    attpool = ctx.enter_context(tc.tile_pool(name="att", bufs=3))
    outpool = ctx.enter_context(tc.tile_pool(name="out", bufs=2))
    rpool = ctx.enter_context(tc.tile_pool(name="rinv", bufs=3))
    # PSUM: 2 x 3-bank big-matmul group tiles + 2 x 1-bank scores tiles
    psA = ctx.enter_context(tc.tile_pool(name="psA", bufs=2, space="PSUM"))
    psS = ctx.enter_context(tc.tile_pool(name="psS", bufs=2, space="PSUM"))

    # Constants: identity for PE transposes, W (fp16 for same-dtype
    # matmuls), and the alpha row expanded to one fp16 weight per
    # (i, t) column so the pooling multiply is unit-stride packed.
    ident = consts.tile([P, P], _F32)
    make_identity(nc, ident)
    ident16 = consts.tile([P, P], _F16)
    make_identity(nc, ident16)
    w_sb = consts.tile([T, T], _F32)
    nc.gpsimd.dma_start(out=w_sb, in_=w_ap)
    w16 = consts.tile([T, T], _F16)
    nc.vector.tensor_copy(w16, w_sb)
    alpha_row = consts.tile([P, N], _F32)
    nc.gpsimd.dma_start(out=alpha_row, in_=alpha_ap[None, :].to_broadcast([P, N]))
    alpha_full = consts.tile([P, F], _F16)
    nc.vector.tensor_copy(
        alpha_full.rearrange("p (i t) -> p i t", t=T),
        alpha_row[:, :, None].to_broadcast([P, N, T]),
    )

    def phase_load(b):
        """One DMA for the whole batch (1.9 MB, SP ring)."""
        x_t = xpool.tile([P, CC, F], _F16, tag="x")
        nc.sync.dma_start(
            out=x_t, in_=x_flat[b].rearrange("(cc p) f -> p cc f", p=P)
        )
        k_c = kpool.tile([P, CC, T], _F32, tag="k")
        return {"x_t": x_t, "k_c": k_c}

    def phase_pool(st, cc):
        """k for one c-chunk, entirely on DVE in packed 2x mode."""
        prod = prodpool.tile([P, F], _F16, tag="prod")
        nc.vector.tensor_mul(prod, st["x_t"][:, cc, :], alpha_full)
        for d0, s0, s1 in _FOLDS:
            n = s1 - s0
            nc.vector.tensor_add(
                prod[:, d0 : d0 + n], prod[:, d0 : d0 + n], prod[:, s0:s1]
            )
        nc.vector.reduce_sum(
            out=st["k_c"][:, cc, :],
            in_=prod[:, : _REM * T].rearrange("p (i t) -> p t i", t=T),
            axis=mybir.AxisListType.X,
        )

    def phase_scores_a(st):
        """Transpose k -> kT (both c-chunks into one PSUM bank)."""
        ps_kt = psS.tile([P, 512], _F32, tag="ps", name="ps_kt")
        nc.tensor.transpose(ps_kt[:T, 0:P], st["k_c"][:, 0, :], ident)
        nc.tensor.transpose(ps_kt[:T, P:C], st["k_c"][:, 1, :], ident)
        kt16 = ktpool.tile([T, C], _F16, tag="kt")
        nc.scalar.copy(out=kt16, in_=ps_kt[:T, :C])
        st["kt16"] = kt16

    def phase_scores_b(st):
        """kWT = W^T kT; scores rows; exp (+denominator via accum)."""
        kt16 = st["kt16"]
        ps_kwt = psS.tile([P, 512], _F32, tag="ps", name="ps_kwt")
        nc.tensor.matmul(
            ps_kwt[:T, :C], lhsT=w16, rhs=kt16, start=True, stop=True
        )
        kwt16 = ktpool.tile([T, C], _F16, tag="kwt")
        nc.scalar.copy(out=kwt16, in_=ps_kwt[:T, :C])
        # scores[c, d] rows: c of this chunk on partitions, d free
        ps_sc = psS.tile([P, 512], _F32, tag="ps", name="ps_sc")
        for cc in range(CC):
            nc.tensor.matmul(
                ps_sc[:, ts(cc, C)],
                lhsT=kwt16[:, ts(cc, P)],
                rhs=kt16,
                start=True,
                stop=True,
            )
        att32 = apool.tile([P, CC, C], _F32, tag="att32")
        den = rpool.tile([P, CC, 2], _F32, tag="den")
        for cc in range(CC):
            nc.scalar.activation(
                out=att32[:, cc, :],
                in_=ps_sc[:, ts(cc, C)],
                func=mybir.ActivationFunctionType.Exp,
                accum_out=den[:, cc, 0:1],
            )
        st["att32"] = att32
        st["den"] = den

    def phase_att_fin(st):
        """Normalize to fp16 weights and transpose to [d-part, c]."""
        att32, den = st["att32"], st["den"]
        nc.vector.reciprocal(out=den[:, :, 1], in_=den[:, :, 0])
        a16r = apool.tile([P, CC, C], _F16, tag="a16r")
        for cc in range(CC):
            nc.scalar.mul(
                out=a16r[:, cc, :], in_=att32[:, cc, :], mul=den[:, cc, 1:2]
            )
        ps_at = psS.tile([P, 512], _F16, tag="ps", name="ps_at")
        for dc in range(CC):
            for cc in range(CC):
                nc.tensor.transpose(
                    ps_at[:, ts(dc * CC + cc, P)],
                    a16r[:, cc, ts(dc, P)],
                    ident16,
                )
        att_t = attpool.tile([P, CC, C], _F16, tag="attT")
        nc.scalar.copy(out=att_t.rearrange("p a c -> p (a c)"), in_=ps_at)
        st["att_t"] = att_t

    def phase_mm_fin(b, st, cc):
        """Big matmul (all fp16) + drains for one c-chunk, grouped in
        3 PSUM group-tiles with one ACT drain copy each."""
        x_t, att_t = st["x_t"], st["att_t"]
        if cc == 0:
            st["o_t"] = outpool.tile([P, CC, F], _F16, tag="o", name="o_t")
        o_t = st["o_t"]
        for gi, (g0, gsz, subs) in enumerate(_GROUPS):
            pt = psA.tile([P, 1536], _F32, tag="ps_mm", name=f"ps_mm{gi}")
            for dc in range(CC):
                s0 = 0
                for ssz in subs:
                    nc.tensor.matmul(
                        pt[:, s0 : s0 + ssz],
                        lhsT=att_t[:, dc, ts(cc, P)],
                        rhs=x_t[:, dc, g0 + s0 : g0 + s0 + ssz],
                        start=(dc == 0),
                        stop=(dc == CC - 1),
                    )
                    s0 += ssz
            nc.scalar.copy(out=o_t[:, cc, g0 : g0 + gsz], in_=pt[:, :gsz])
        nc.scalar.dma_start(out=out_flat[b, ts(cc, P), :], in_=o_t[:, cc, :])

    # Three-stage pipeline, one-step load prefetch.
    states = {}
    for s in range(-1, B_LOC + 2):
        if 0 <= s + 1 < B_LOC:
            states[s + 1] = phase_load(s + 1)
        st2 = states.get(s - 2)
        st1 = states.get(s - 1)
        st0 = states.get(s)
        if st2 is not None:
            phase_mm_fin(s - 2, st2, 0)
        if st1 is not None:
            phase_scores_a(st1)
        if st0 is not None:
            phase_pool(st0, 0)
        if st2 is not None:
            phase_mm_fin(s - 2, st2, 1)
            states.pop(s - 2)
        if st1 is not None:
            phase_scores_b(st1)
        if st0 is not None:
            phase_pool(st0, 1)
        if st1 is not None:
            phase_att_fin(st1)
    ctx.close()


_CACHED_NC = None


def _build():
    global _CACHED_NC
    if _CACHED_NC is not None:
        return _CACHED_NC
    nc = bacc.Bacc("TRN2", target_bir_lowering=False, debug=False, num_devices=NCORES)
    x_d = nc.dram_tensor("x", [B_LOC, C, N, T], _F16, kind="ExternalInput").ap()
    w_d = nc.dram_tensor("W", [T, T], _F32, kind="ExternalInput").ap()
    a_d = nc.dram_tensor("alpha", [N], _F32, kind="ExternalInput").ap()
    o_d = nc.dram_tensor("out", [B_LOC, C, N, T], _F16, kind="ExternalOutput").ap()
    with tile.TileContext(nc) as tc:
        _emit_core_kernel(tc, x_d, w_d, a_d, o_d)
    nc.compile()
    _CACHED_NC = nc
    return nc


def run(x, W, alpha, trace=False, **spmd_kwargs):
    """Run on 8 cores; returns (full output [B,C,N,T], BassKernelResults)."""
    x = np.ascontiguousarray(np.asarray(x, dtype=np.float32))
    W = np.ascontiguousarray(np.asarray(W, dtype=np.float32))
    alpha = np.ascontiguousarray(np.asarray(alpha, dtype=np.float32))
    assert x.shape == (B, C, N, T) and W.shape == (T, T) and alpha.shape == (N,)

    x16 = x.astype(np.float16)
    nc = _build()
    in_maps = [
        {"x": x16[i * B_LOC : (i + 1) * B_LOC], "W": W, "alpha": alpha}
        for i in range(NCORES)
    ]
    res = run_bass_kernel_spmd(
        nc, in_maps, core_ids=list(range(NCORES)), trace=trace, **spmd_kwargs
    )
    out = np.concatenate([r["out"] for r in res.results], axis=0).astype(np.float32)
    return out, res


def kernel(x, W, alpha):
    out, _ = run(x, W, alpha)
    return out


# revision 10
# speedup vs baseline: 1.7153x; 1.0698x over previous
"""Trainium2 Bass kernel for nn_CAttention (channel attention).

Reference computation (per batch b):
    k      = einsum('cit,i->ct', x[b], alpha)          # [C, T]
    scores = k @ W @ k.T                               # [C, C]
    att    = softmax(scores, axis=-1)
    out[b] = att @ x[b].reshape(C, N*T)                # [C, N*T]

Shapes (hardcoded): x [64, 256, 307, 12], W [12, 12], alpha [307].
Sharding: data-parallel over batch B across 8 cores (8 batches/core);
W and alpha replicated.

The kernel is HBM-DMA bound, so x and out travel as fp16 (host casts
f32->fp16 in, fp16->f32 back): 15.1 MB in + 15.1 MB out per core =>
~85 us floor at the ~350 GB/s effective per-core DMA rate.  fp16
everywhere measures ~1.5e-3 L2 end-to-end (bf16 weights measured
1.3e-2 on HW - too close to the gate; and NOTE: a mixed bf16 x fp16
matmul FAULTS the device, NRT_EXEC_UNIT_UNRECOVERABLE, so every
matmul here keeps both operands the same dtype).

exp(scores) reaches e^31 which overflows fp16, so attention weights
are normalized BEFORE they become matmul weights: scores are computed
in row orientation [c-part, d-free] (stationary kWT chunk x moving
kT), exp on ACT emits f32 weights plus the softmax denominator via
accum_out in the same pass, DVE takes one reciprocal, ACT's normalize
multiply (per-partition rinv) emits fp16 weights <= 1, and four PE
transposes flip them to the [d-part, c] stationary layout.

Per-engine budget per batch (target period ~10.5 us):
 - DMA: one 1.9 MB batch load (SP ring) + two 0.95 MB stores (ACT).
 - DVE ~9.9: owns k-pooling: packed-2x fp16 multiply by a materialized
   alpha-per-(i,t) row, in-place packed fold tree 307->154->77->39->20
   i's, one strided reduce_sum.  (TensorReduce/TensorTensorReduce have
   no DVE fast modes, and GpSimd tensor ops run at 0.42 efficiency and
   contend for DVE's tensor_tensor port, so all-DVE packed is optimal;
   the Pool engine stays idle on purpose.)  Plus the one reciprocal.
 - ACT ~9.9: PSUM drains as 3 copies per c-chunk (1536/1536/612 cols
   spanning 3/3/2 banks - fewer, larger copies; ACT gets no 16-bit
   speedup so element count and instruction count are what matter),
   kT/kWT evacuations (casting to fp16), exp+accum, normalize.
 - PE ~7.5: big matmul all-fp16 at 1 col/cyc, fp16 scores matmuls.
 - PSUM: two 3-bank tiles (big-matmul groups, ping-pong) + two 1-bank
   tiles (scores chain) = exactly 8 banks, no pool conflicts.

Three-stage pipeline with one-step load prefetch; the emission order
interleaves engines so every cross-engine hop has queued work and the
DVE order is pool(c0), pool(c1), recip.
"""

from contextlib import ExitStack

import numpy as np

import concourse.bass as bass
import concourse.bass_utils as _bass_utils
import concourse.tile as tile
from concourse import bacc, mybir
from concourse.bass import ts
from concourse.bass_utils import run_bass_kernel_spmd
from concourse.masks import make_identity

B, C, N, T = 64, 256, 307, 12
NCORES = 8
B_LOC = B // NCORES          # 8 batches per core
F = N * T                    # 3684 flattened free dim
P = 128                      # partitions
CC = C // P                  # 2 c-chunks

# Big-matmul PSUM groups per c-chunk: (f0, size, [sub-tile sizes]).
# Each sub-tile is one matmul dest (<=512 cols, bank-aligned inside a
# 3-bank group tile); each group drains with a single ACT copy.
_GROUPS = [
    (0, 1536, (512, 512, 512)),
    (1536, 1536, (512, 512, 512)),
    (3072, 612, (512, 100)),
]

# In-place fold tree for the i-reduction, in element offsets (i*T).
# Each level folds src range [s0, s1) onto dst [d0, d0+(s1-s0)); all
# ranges are 4B-aligned with even element counts so fp16 tensor_add
# runs in packed 2x_1p mode.  Afterwards i in [0, 20) remains.
_FOLDS = [
    (12, 1848, 3684),   # i[154..307) -> i[1..154)
    (0, 924, 1848),     # i[77..154)  -> i[0..77)
    (12, 468, 924),     # i[39..77)   -> i[1..39)
    (12, 240, 468),     # i[20..39)   -> i[1..20)
]
_REM = 20                    # i's remaining for the final reduce_sum

_F32 = mybir.dt.float32
_F16 = mybir.dt.float16


def _emit_core_kernel(tc, x_ap, w_ap, alpha_ap, out_ap):
    """Emit the per-core program. x_ap/out_ap: [B_LOC, C, N, T] DRAM fp16."""
    nc = tc.nc
    ctx = ExitStack()

    x_flat = x_ap.rearrange("b c i t -> b c (i t)")      # [B_LOC, C, F]
    out_flat = out_ap.rearrange("b c i t -> b c (i t)")  # [B_LOC, C, F]

    consts = ctx.enter_context(tc.tile_pool(name="consts", bufs=1))
    xpool = ctx.enter_context(tc.tile_pool(name="x", bufs=5))
    prodpool = ctx.enter_context(tc.tile_pool(name="prod", bufs=2))
    kpool = ctx.enter_context(tc.tile_pool(name="k", bufs=3))
    ktpool = ctx.enter_context(tc.tile_pool(name="kt", bufs=3))
    apool = ctx.enter_context(tc.tile_pool(name="att32", bufs=2))
    attpool = ctx.enter_context(tc.tile_pool(name="att", bufs=4))
    outpool = ctx.enter_context(tc.tile_pool(name="out", bufs=2))
    rpool = ctx.enter_context(tc.tile_pool(name="rinv", bufs=3))
    # PSUM: 2 x 3-bank big-matmul group tiles + 2 x 1-bank scores tiles
    psA = ctx.enter_context(tc.tile_pool(name="psA", bufs=2, space="PSUM"))
    psS = ctx.enter_context(tc.tile_pool(name="psS", bufs=2, space="PSUM"))

    # Constants: identity for PE transposes, W (fp16 for same-dtype
    # matmuls), and the alpha row expanded to one fp16 weight per
    # (i, t) column so the pooling multiply is unit-stride packed.
    ident = consts.tile([P, P], _F32)
    make_identity(nc, ident)
    ident16 = consts.tile([P, P], _F16)
    make_identity(nc, ident16)
    w_sb = consts.tile([T, T], _F32)
    nc.gpsimd.dma_start(out=w_sb, in_=w_ap)
    w16 = consts.tile([T, T], _F16)
    nc.vector.tensor_copy(w16, w_sb)
    alpha_row = consts.tile([P, N], _F32)
    nc.gpsimd.dma_start(out=alpha_row, in_=alpha_ap[None, :].to_broadcast([P, N]))
    alpha_full = consts.tile([P, F], _F16)
    nc.vector.tensor_copy(
        alpha_full.rearrange("p (i t) -> p i t", t=T),
        alpha_row[:, :, None].to_broadcast([P, N, T]),
    )

    def phase_load(b):
        """One DMA for the whole batch (1.9 MB, SP ring)."""
        x_t = xpool.tile([P, CC, F], _F16, tag="x")
        nc.sync.dma_start(
            out=x_t, in_=x_flat[b].rearrange("(cc p) f -> p cc f", p=P)
        )
        k_c = kpool.tile([P, CC, T], _F32, tag="k")
        return {"x_t": x_t, "k_c": k_c}

    def phase_pool(st, cc):
        """k for one c-chunk, entirely on DVE in packed 2x mode."""
        prod = prodpool.tile([P, F], _F16, tag="prod")
        nc.vector.tensor_mul(prod, st["x_t"][:, cc, :], alpha_full)
        for d0, s0, s1 in _FOLDS:
            n = s1 - s0
            nc.vector.tensor_add(
                prod[:, d0 : d0 + n], prod[:, d0 : d0 + n], prod[:, s0:s1]
            )
        nc.vector.reduce_sum(
            out=st["k_c"][:, cc, :],
            in_=prod[:, : _REM * T].rearrange("p (i t) -> p t i", t=T),
            axis=mybir.AxisListType.X,
        )

    def phase_scores_a(st):
        """Transpose k -> kT (both c-chunks into one PSUM bank)."""
        ps_kt = psS.tile([P, 512], _F32, tag="ps", name="ps_kt")
        nc.tensor.transpose(ps_kt[:T, 0:P], st["k_c"][:, 0, :], ident)
        nc.tensor.transpose(ps_kt[:T, P:C], st["k_c"][:, 1, :], ident)
        kt16 = ktpool.tile([T, C], _F16, tag="kt")
        nc.scalar.copy(out=kt16, in_=ps_kt[:T, :C])
        st["kt16"] = kt16

    def phase_scores_b(st):
        """kWT = W^T kT; scores rows; exp (+denominator via accum)."""
        kt16 = st["kt16"]
        ps_kwt = psS.tile([P, 512], _F32, tag="ps", name="ps_kwt")
        nc.tensor.matmul(
            ps_kwt[:T, :C], lhsT=w16, rhs=kt16, start=True, stop=True
        )
        kwt16 = ktpool.tile([T, C], _F16, tag="kwt")
        nc.scalar.copy(out=kwt16, in_=ps_kwt[:T, :C])
        # scores[c, d] rows: c of this chunk on partitions, d free
        ps_sc = psS.tile([P, 512], _F32, tag="ps", name="ps_sc")
        for cc in range(CC):
            nc.tensor.matmul(
                ps_sc[:, ts(cc, C)],
                lhsT=kwt16[:, ts(cc, P)],
                rhs=kt16,
                start=True,
                stop=True,
            )
        att32 = apool.tile([P, CC, C], _F32, tag="att32")
        den = rpool.tile([P, CC, 2], _F32, tag="den")
        for cc in range(CC):
            nc.scalar.activation(
                out=att32[:, cc, :],
                in_=ps_sc[:, ts(cc, C)],
                func=mybir.ActivationFunctionType.Exp,
                accum_out=den[:, cc, 0:1],
            )
        st["att32"] = att32
        st["den"] = den

    def phase_att_fin(st):
        """Normalize to fp16 weights and transpose to [d-part, c]."""
        att32, den = st["att32"], st["den"]
        nc.vector.reciprocal(out=den[:, :, 1], in_=den[:, :, 0])
        a16r = apool.tile([P, CC, C], _F16, tag="a16r")
        for cc in range(CC):
            nc.scalar.mul(
                out=a16r[:, cc, :], in_=att32[:, cc, :], mul=den[:, cc, 1:2]
            )
        ps_at = psS.tile([P, 512], _F16, tag="ps", name="ps_at")
        for dc in range(CC):
            for cc in range(CC):
                nc.tensor.transpose(
                    ps_at[:, ts(dc * CC + cc, P)],
                    a16r[:, cc, ts(dc, P)],
                    ident16,
                )
        att_t = attpool.tile([P, CC, C], _F16, tag="attT")
        nc.scalar.copy(out=att_t.rearrange("p a c -> p (a c)"), in_=ps_at)
        st["att_t"] = att_t

    def phase_mm_fin(b, st, cc):
        """Big matmul (all fp16) + drains for one c-chunk, grouped in
        3 PSUM group-tiles with one ACT drain copy each."""
        x_t, att_t = st["x_t"], st["att_t"]
        if cc == 0:
            st["o_t"] = outpool.tile([P, CC, F], _F16, tag="o", name="o_t")
        o_t = st["o_t"]
        for gi, (g0, gsz, subs) in enumerate(_GROUPS):
            pt = psA.tile([P, 1536], _F32, tag="ps_mm", name=f"ps_mm{gi}")
            for dc in range(CC):
                s0 = 0
                for ssz in subs:
                    nc.tensor.matmul(
                        pt[:, s0 : s0 + ssz],
                        lhsT=att_t[:, dc, ts(cc, P)],
                        rhs=x_t[:, dc, g0 + s0 : g0 + s0 + ssz],
                        start=(dc == 0),
                        stop=(dc == CC - 1),
                    )
                    s0 += ssz
            nc.scalar.copy(out=o_t[:, cc, g0 : g0 + gsz], in_=pt[:, :gsz])
        nc.scalar.dma_start(out=out_flat[b, ts(cc, P), :], in_=o_t[:, cc, :])

    # Four-stage pipeline, one-step load prefetch: attention weights
    # for batch s are final one full period before mm(s) needs them, so
    # the serial exp->recip->normalize->transpose tail never stalls the
    # PE stream (which also keeps the HAM clock gate warm).
    states = {}
    for s in range(-1, B_LOC + 3):
        if 0 <= s + 1 < B_LOC:
            states[s + 1] = phase_load(s + 1)
        st3 = states.get(s - 3)
        st1 = states.get(s - 1)
        st0 = states.get(s)
        if st3 is not None:
            phase_mm_fin(s - 3, st3, 0)
        if st1 is not None:
            phase_scores_a(st1)
        if st0 is not None:
            phase_pool(st0, 0)
        if st3 is not None:
            phase_mm_fin(s - 3, st3, 1)
            states.pop(s - 3)
        if st1 is not None:
            phase_scores_b(st1)
        if st0 is not None:
            phase_pool(st0, 1)
        if st1 is not None:
            phase_att_fin(st1)
    ctx.close()


_CACHED_NC = None


def _build():
    global _CACHED_NC
    if _CACHED_NC is not None:
        return _CACHED_NC
    nc = bacc.Bacc("TRN2", target_bir_lowering=False, debug=False, num_devices=NCORES)
    x_d = nc.dram_tensor("x", [B_LOC, C, N, T], _F16, kind="ExternalInput").ap()
    w_d = nc.dram_tensor("W", [T, T], _F32, kind="ExternalInput").ap()
    a_d = nc.dram_tensor("alpha", [N], _F32, kind="ExternalInput").ap()
    o_d = nc.dram_tensor("out", [B_LOC, C, N, T], _F16, kind="ExternalOutput").ap()
    with tile.TileContext(nc) as tc:
        _emit_core_kernel(tc, x_d, w_d, a_d, o_d)
    nc.compile()
    _CACHED_NC = nc
    return nc


def run(x, W, alpha, trace=False, **spmd_kwargs):
    """Run on 8 cores; returns (full output [B,C,N,T], BassKernelResults)."""
    x = np.ascontiguousarray(np.asarray(x, dtype=np.float32))
    W = np.ascontiguousarray(np.asarray(W, dtype=np.float32))
    alpha = np.ascontiguousarray(np.asarray(alpha, dtype=np.float32))
    assert x.shape == (B, C, N, T) and W.shape == (T, T) and alpha.shape == (N,)

    x16 = x.astype(np.float16)
    nc = _build()
    in_maps = [
        {"x": x16[i * B_LOC : (i + 1) * B_LOC], "W": W, "alpha": alpha}
        for i in range(NCORES)
    ]
    res = run_bass_kernel_spmd(
        nc, in_maps, core_ids=list(range(NCORES)), trace=trace, **spmd_kwargs
    )
    out = np.concatenate([r["out"] for r in res.results], axis=0).astype(np.float32)
    return out, res


def kernel(x, W, alpha):
    out, _ = run(x, W, alpha)
    return out
